# revision 4
# baseline (speedup 1.0000x reference)
"""Trainium2 Bass kernel for nn_CausalTrajectoryPrediction.

Math (per node n, from the reference):
  A1[n,h]  = <W1[n,h,:], x> - x_n * W1[n,h,n]        (x with x_n zeroed)
  r1       = relu(A1)
  r2[n,m]  = relu(<W2[n,m,:], r1>)
  A3[n,k]  = <W3[n,k,:256], r2> + x_n * W3[n,k,256+n] + b3[n,k]
  h3       = relu(A3)
  d[n]     = relu(<W4[n,0,:], h3> + b4[n])
Only W3[:, :, :256] plus its per-node diagonal column is ever used, so the
second half of W3 (minus the diagonal) is never read from HBM.

Sharding: nodes 32*c..32*c+32 on core c (expert parallel). Host-side prep is
layout-only: slicing, transposes (so the contraction index lands on SBUF
partitions), and packing of the tiny per-node vectors. All FLOPs run on
device: each stage is a chain of accumulating 128x128 @ 128x1 PE matvecs.
"""

import numpy as np

N_CORES = 8
N, H, M = 256, 1024, 256
NPC = N // N_CORES  # 32 nodes per core

_module_cache = {}


def _build_module(npc, mm_dtype_name="float32r"):
    import concourse.bacc as bacc
    import concourse.tile as tile
    from concourse import mybir

    f32 = mybir.dt.float32
    mmdt = getattr(mybir.dt, mm_dtype_name)
    AF = mybir.ActivationFunctionType
    OP = mybir.AluOpType

    nc = bacc.Bacc("TRN2", target_bir_lowering=False, debug=False)

    w1t = nc.dram_tensor("w1t", [npc, 256, 1024], mmdt, kind="ExternalInput")
    w2t = nc.dram_tensor("w2t", [npc, 1024, 256], mmdt, kind="ExternalInput")
    w3t = nc.dram_tensor("w3t", [npc, 256, 1024], mmdt, kind="ExternalInput")
    aux = nc.dram_tensor("aux", [npc, 128, 32], f32, kind="ExternalInput")
    xc = nc.dram_tensor("xc", [128, 3], mmdt, kind="ExternalInput")
    xn = nc.dram_tensor("xn", [1, npc], f32, kind="ExternalInput")
    b4s = nc.dram_tensor("b4s", [npc, 1], f32, kind="ExternalInput")
    out = nc.dram_tensor("out", [npc, 1], f32, kind="ExternalOutput")

    with tile.TileContext(nc) as tc:
        with (
            tc.tile_pool(name="singles", bufs=1) as singles,
            tc.tile_pool(name="wpool", bufs=4) as wpool,
            tc.tile_pool(name="auxp", bufs=4) as auxp,
            tc.tile_pool(name="vec", bufs=4) as vec,
            tc.tile_pool(name="psum", bufs=2, space="PSUM") as psum,
            tc.tile_pool(name="psum_d", bufs=1, space="PSUM") as psum_d,
        ):
            xc_sb = singles.tile([128, 3], mmdt)
            nc.sync.dma_start(out=xc_sb[:], in_=xc[:, :])

            # broadcast x_n values across all partitions: [128, npc]
            import concourse.bass as bass

            xn_ap = xn[:, :]
            xn_b = bass.AP(
                tensor=xn_ap.tensor,
                offset=xn_ap.offset,
                ap=[[0, 128]] + [list(d) for d in xn_ap.ap[1:]],
            )
            xnb = singles.tile([128, npc], f32)
            nc.gpsimd.dma_start(out=xnb[:], in_=xn_b)
            xnegb = singles.tile([128, npc], f32)
            nc.vector.tensor_scalar_mul(out=xnegb[:], in0=xnb[:], scalar1=-1.0)

            ones_col = singles.tile([128, 2], f32)
            nc.vector.memset(ones_col[:], 1.0)
            zero1 = singles.tile([128, 1], f32)
            nc.vector.memset(zero1[:], 0.0)
            b4sb = singles.tile([npc, 1], f32)
            nc.scalar.dma_start(out=b4sb[:], in_=b4s[:, :])
            pp = singles.tile([128, npc], f32)

            for l in range(npc):
                w1 = wpool.tile([128, 2, 1024], mmdt, tag="w1")
                nc.sync.dma_start(
                    out=w1[:], in_=w1t[l, :, :].rearrange("(q p) h -> p q h", p=128)
                )
                w2 = wpool.tile([128, 8, 256], mmdt, tag="w2")
                nc.sync.dma_start(
                    out=w2[:], in_=w2t[l, :, :].rearrange("(t p) m -> p t m", p=128)
                )
                w3 = wpool.tile([128, 2, 1024], mmdt, tag="w3")
                nc.sync.dma_start(
                    out=w3[:], in_=w3t[l, :, :].rearrange("(q p) k -> p q k", p=128)
                )
                ax = auxp.tile([128, 32], f32, tag="ax")
                nc.scalar.dma_start(out=ax[:], in_=aux[l, :, :])

                # S1: A1 chunks t: sum over j-chunks q
                a1p = psum.tile([128, 8, 2], f32, tag="a1")
                for t in range(8):
                    for q in range(2):
                        nc.tensor.matmul(
                            out=a1p[:, t, :],
                            lhsT=w1[:, q, t * 128 : (t + 1) * 128],
                            rhs=xc_sb[:, q : q + 2],
                            start=(q == 0),
                            stop=(q == 1),
                        )
                # a1s = a1p - x_n * w1diag ; relu
                a1s = vec.tile([128, 8], f32, tag="a1s")
                nc.vector.tensor_scalar_mul(
                    out=a1s[:], in0=ax[:, 0:8], scalar1=xnegb[:, l : l + 1]
                )
                nc.vector.tensor_add(out=a1s[:], in0=a1s[:], in1=a1p[:, :, 0])
                r1c = vec.tile([128, 9], mmdt, tag="r1c")
                nc.scalar.activation(out=r1c[:, 0:8], in_=a1s[:], func=AF.Relu)
                nc.scalar.activation(out=r1c[:, 8:9], in_=zero1[:], func=AF.Relu)

                # S2: r2 chunks q: sum over h-chunks t
                a2p = psum.tile([128, 2, 2], f32, tag="a2")
                for q in range(2):
                    for t in range(8):
                        nc.tensor.matmul(
                            out=a2p[:, q, :],
                            lhsT=w2[:, t, q * 128 : (q + 1) * 128],
                            rhs=r1c[:, t : t + 2],
                            start=(t == 0),
                            stop=(t == 7),
                        )
                r2c = vec.tile([128, 3], mmdt, tag="r2c")
                nc.scalar.activation(out=r2c[:, 0:2], in_=a2p[:, :, 0], func=AF.Relu)
                nc.scalar.activation(out=r2c[:, 2:3], in_=zero1[:], func=AF.Relu)

                # S3: A3 chunks t: sum over m-chunks q
                a3p = psum.tile([128, 8, 2], f32, tag="a3")
                for t in range(8):
                    for q in range(2):
                        nc.tensor.matmul(
                            out=a3p[:, t, :],
                            lhsT=w3[:, q, t * 128 : (t + 1) * 128],
                            rhs=r2c[:, q : q + 2],
                            start=(q == 0),
                            stop=(q == 1),
                        )
                # h3 = relu(a3p + x_n * w3diag + b3)
                a3s = vec.tile([128, 8], f32, tag="a3s")
                nc.vector.tensor_scalar_mul(
                    out=a3s[:], in0=ax[:, 8:16], scalar1=xnb[:, l : l + 1]
                )
                nc.vector.tensor_add(out=a3s[:], in0=a3s[:], in1=a3p[:, :, 0])
                nc.vector.tensor_add(out=a3s[:], in0=a3s[:], in1=ax[:, 16:24])
                h3 = vec.tile([128, 8], f32, tag="h3")
                nc.scalar.activation(out=h3[:], in_=a3s[:], func=AF.Relu)

                # S4 partial dot: pp[:, l] = sum_f w4t * h3 (per partition)
                t4 = vec.tile([128, 8], f32, tag="t4")
                nc.vector.tensor_mul(out=t4[:], in0=ax[:, 24:32], in1=h3[:])
                nc.vector.tensor_reduce(
                    pp[:, l : l + 1], t4[:], mybir.AxisListType.X, OP.add
                )

            # d = relu(colsum(pp) + b4)
            dp = psum_d.tile([npc, 2], f32, tag="d")
            nc.tensor.matmul(
                out=dp[:], lhsT=pp[:, 0:npc], rhs=ones_col[:], start=True, stop=True
            )
            ds = vec.tile([npc, 1], f32, tag="ds")
            nc.vector.tensor_add(out=ds[:], in0=dp[:, 0:1], in1=b4sb[:])
            nc.scalar.activation(out=ds[:], in_=ds[:], func=AF.Relu)
            nc.sync.dma_start(out=out[:, :], in_=ds[:])

    nc.compile()
    return nc


def _get_module(npc=NPC, mm_dtype_name="float32r"):
    key = (npc, mm_dtype_name)
    if key not in _module_cache:
        _module_cache[key] = _build_module(npc, mm_dtype_name)
    return _module_cache[key]


def _prep_in_maps(x, W1, W2, W3, b3, W4, b4, npc=NPC):
    """Layout-only host prep: slice per core, transpose so the contraction
    index is the SBUF partition dim, pack per-node small vectors."""
    x = np.asarray(x, np.float32).reshape(1, N)
    W1 = np.asarray(W1, np.float32)
    W2 = np.asarray(W2, np.float32)
    W3 = np.asarray(W3, np.float32)
    b3 = np.asarray(b3, np.float32)
    W4 = np.asarray(W4, np.float32)
    b4 = np.asarray(b4, np.float32).reshape(N, 1)

    ar = np.arange(N)
    W1T = np.ascontiguousarray(W1.transpose(0, 2, 1))  # [N, 256, 1024]
    W2T = np.ascontiguousarray(W2.transpose(0, 2, 1))  # [N, 1024, 256]
    W3T = np.ascontiguousarray(W3[:, :, :M].transpose(0, 2, 1))  # [N, 256, 1024]
    w1d = W1[ar, :, ar]  # [N, 1024]
    w3d = W3[ar, :, M + ar]  # [N, 1024]
    w4s = W4[:, 0, :]  # [N, 1024]

    def colmajor8(a):  # [n, 1024] -> [n, 128, 8] with (p, t) = a[:, t*128+p]
        return a.reshape(-1, 8, 128).transpose(0, 2, 1)

    aux = np.ascontiguousarray(
        np.concatenate(
            [colmajor8(w1d), colmajor8(w3d), colmajor8(b3), colmajor8(w4s)], axis=2
        ),
        dtype=np.float32,
    )  # [N, 128, 32]
    xcv = np.zeros((128, 3), np.float32)
    xcv[:, 0:2] = x.reshape(2, 128).T

    n_cores_used = N // npc
    in_maps = []
    for c in range(n_cores_used):
        sl = slice(npc * c, npc * (c + 1))
        in_maps.append(
            {
                "w1t": W1T[sl],
                "w2t": W2T[sl],
                "w3t": W3T[sl],
                "aux": aux[sl],
                "xc": xcv,
                "xn": np.ascontiguousarray(x[:, sl]),
                "b4s": np.ascontiguousarray(b4[sl]),
            }
        )
    return in_maps


def kernel(x, W1, W2, W3, b3, W4, b4, t=0, **_unused):
    from concourse.bass_utils import run_bass_kernel_spmd

    nc = _get_module()
    in_maps = _prep_in_maps(x, W1, W2, W3, b3, W4, b4)
    res = run_bass_kernel_spmd(nc, in_maps, core_ids=list(range(N_CORES)))
    out = np.concatenate([res.results[c]["out"][:, 0] for c in range(N_CORES)])
    kernel.last_results = res
    return np.ascontiguousarray(out.reshape(1, N)).astype(np.float32)


# revision 5
# speedup vs baseline: 2.5090x; 2.5090x over previous
"""Trainium2 Bass kernel for nn_CausalTrajectoryPrediction.

Math (per node n, from the reference):
  A1[n,h]  = <W1[n,h,:], x> - x_n * W1[n,h,n]        (x with x_n zeroed)
  r1       = relu(A1)
  r2[n,m]  = relu(<W2[n,m,:], r1>)
  A3[n,k]  = <W3[n,k,:256], r2> + x_n * W3[n,k,256+n] + b3[n,k]
  h3       = relu(A3)
  d[n]     = relu(<W4[n,0,:], h3> + b4[n])
Only W3[:, :, :256] plus its per-node diagonal column is ever used, so the
second half of W3 (minus the diagonal) is never read from HBM.

Sharding: nodes 32*c..32*c+32 on core c (expert parallel). Host-side prep is
layout-only: slicing, transposes (so the contraction index lands on SBUF
partitions), and packing of the tiny per-node vectors. All FLOPs run on
device: each stage is a chain of accumulating 128x128 @ 128x1 PE matvecs.
"""

import numpy as np

N_CORES = 8
N, H, M = 256, 1024, 256
NPC = N // N_CORES  # 32 nodes per core

_module_cache = {}


def _build_module(npc, mm_dtype_name="float16"):
    import concourse.bacc as bacc
    import concourse.tile as tile
    from concourse import mybir

    f32 = mybir.dt.float32
    mmdt = getattr(mybir.dt, mm_dtype_name)
    AF = mybir.ActivationFunctionType
    OP = mybir.AluOpType

    nc = bacc.Bacc("TRN2", target_bir_lowering=False, debug=False)

    w1t = nc.dram_tensor("w1t", [npc, 256, 1024], mmdt, kind="ExternalInput")
    w2t = nc.dram_tensor("w2t", [npc, 1024, 256], mmdt, kind="ExternalInput")
    w3t = nc.dram_tensor("w3t", [npc, 256, 1024], mmdt, kind="ExternalInput")
    aux = nc.dram_tensor("aux", [npc, 128, 32], f32, kind="ExternalInput")
    xc = nc.dram_tensor("xc", [128, 3], mmdt, kind="ExternalInput")
    xn = nc.dram_tensor("xn", [1, npc], f32, kind="ExternalInput")
    b4s = nc.dram_tensor("b4s", [npc, 1], f32, kind="ExternalInput")
    out = nc.dram_tensor("out", [npc, 1], f32, kind="ExternalOutput")

    with tile.TileContext(nc) as tc:
        with (
            tc.tile_pool(name="singles", bufs=1) as singles,
            tc.tile_pool(name="wpool", bufs=4) as wpool,
            tc.tile_pool(name="auxp", bufs=4) as auxp,
            tc.tile_pool(name="vec", bufs=4) as vec,
            tc.tile_pool(name="psum", bufs=2, space="PSUM") as psum,
            tc.tile_pool(name="psum_d", bufs=1, space="PSUM") as psum_d,
        ):
            xc_sb = singles.tile([128, 3], mmdt)
            nc.sync.dma_start(out=xc_sb[:], in_=xc[:, :])

            # broadcast x_n values across all partitions: [128, npc]
            import concourse.bass as bass

            xn_ap = xn[:, :]
            xn_b = bass.AP(
                tensor=xn_ap.tensor,
                offset=xn_ap.offset,
                ap=[[0, 128]] + [list(d) for d in xn_ap.ap[1:]],
            )
            xnb = singles.tile([128, npc], f32)
            nc.gpsimd.dma_start(out=xnb[:], in_=xn_b)
            xnegb = singles.tile([128, npc], f32)
            nc.vector.tensor_scalar_mul(out=xnegb[:], in0=xnb[:], scalar1=-1.0)

            ones_col = singles.tile([128, 2], f32)
            nc.vector.memset(ones_col[:], 1.0)
            zero1 = singles.tile([128, 1], f32)
            nc.vector.memset(zero1[:], 0.0)
            b4sb = singles.tile([npc, 1], f32)
            nc.scalar.dma_start(out=b4sb[:], in_=b4s[:, :])
            pp = singles.tile([128, npc], f32)

            for l in range(npc):
                w1 = wpool.tile([128, 2, 1024], mmdt, tag="w1")
                nc.sync.dma_start(
                    out=w1[:], in_=w1t[l, :, :].rearrange("(q p) h -> p q h", p=128)
                )
                w2 = wpool.tile([128, 8, 256], mmdt, tag="w2")
                nc.sync.dma_start(
                    out=w2[:], in_=w2t[l, :, :].rearrange("(t p) m -> p t m", p=128)
                )
                w3 = wpool.tile([128, 2, 1024], mmdt, tag="w3")
                nc.sync.dma_start(
                    out=w3[:], in_=w3t[l, :, :].rearrange("(q p) k -> p q k", p=128)
                )
                ax = auxp.tile([128, 32], f32, tag="ax")
                nc.scalar.dma_start(out=ax[:], in_=aux[l, :, :])

                # S1: A1 chunks t: sum over j-chunks q
                a1p = psum.tile([128, 8, 2], f32, tag="a1")
                for t in range(8):
                    for q in range(2):
                        nc.tensor.matmul(
                            out=a1p[:, t, :],
                            lhsT=w1[:, q, t * 128 : (t + 1) * 128],
                            rhs=xc_sb[:, q : q + 2],
                            start=(q == 0),
                            stop=(q == 1),
                        )
                # a1s = a1p - x_n * w1diag ; relu
                a1s = vec.tile([128, 8], f32, tag="a1s")
                nc.vector.tensor_scalar_mul(
                    out=a1s[:], in0=ax[:, 0:8], scalar1=xnegb[:, l : l + 1]
                )
                nc.vector.tensor_add(out=a1s[:], in0=a1s[:], in1=a1p[:, :, 0])
                r1c = vec.tile([128, 9], mmdt, tag="r1c")
                nc.scalar.activation(out=r1c[:, 0:8], in_=a1s[:], func=AF.Relu)
                nc.scalar.activation(out=r1c[:, 8:9], in_=zero1[:], func=AF.Relu)

                # S2: r2 chunks q: sum over h-chunks t
                a2p = psum.tile([128, 2, 2], f32, tag="a2")
                for q in range(2):
                    for t in range(8):
                        nc.tensor.matmul(
                            out=a2p[:, q, :],
                            lhsT=w2[:, t, q * 128 : (q + 1) * 128],
                            rhs=r1c[:, t : t + 2],
                            start=(t == 0),
                            stop=(t == 7),
                        )
                r2c = vec.tile([128, 3], mmdt, tag="r2c")
                nc.scalar.activation(out=r2c[:, 0:2], in_=a2p[:, :, 0], func=AF.Relu)
                nc.scalar.activation(out=r2c[:, 2:3], in_=zero1[:], func=AF.Relu)

                # S3: A3 chunks t: sum over m-chunks q
                a3p = psum.tile([128, 8, 2], f32, tag="a3")
                for t in range(8):
                    for q in range(2):
                        nc.tensor.matmul(
                            out=a3p[:, t, :],
                            lhsT=w3[:, q, t * 128 : (t + 1) * 128],
                            rhs=r2c[:, q : q + 2],
                            start=(q == 0),
                            stop=(q == 1),
                        )
                # h3 = relu(a3p + x_n * w3diag + b3)
                a3s = vec.tile([128, 8], f32, tag="a3s")
                nc.vector.tensor_scalar_mul(
                    out=a3s[:], in0=ax[:, 8:16], scalar1=xnb[:, l : l + 1]
                )
                nc.vector.tensor_add(out=a3s[:], in0=a3s[:], in1=a3p[:, :, 0])
                nc.vector.tensor_add(out=a3s[:], in0=a3s[:], in1=ax[:, 16:24])
                h3 = vec.tile([128, 8], f32, tag="h3")
                nc.scalar.activation(out=h3[:], in_=a3s[:], func=AF.Relu)

                # S4 partial dot: pp[:, l] = sum_f w4t * h3 (per partition)
                t4 = vec.tile([128, 8], f32, tag="t4")
                nc.vector.tensor_mul(out=t4[:], in0=ax[:, 24:32], in1=h3[:])
                nc.vector.tensor_reduce(
                    pp[:, l : l + 1], t4[:], mybir.AxisListType.X, OP.add
                )

            # d = relu(colsum(pp) + b4)
            dp = psum_d.tile([npc, 2], f32, tag="d")
            nc.tensor.matmul(
                out=dp[:], lhsT=pp[:, 0:npc], rhs=ones_col[:], start=True, stop=True
            )
            ds = vec.tile([npc, 1], f32, tag="ds")
            nc.vector.tensor_add(out=ds[:], in0=dp[:, 0:1], in1=b4sb[:])
            nc.scalar.activation(out=ds[:], in_=ds[:], func=AF.Relu)
            nc.sync.dma_start(out=out[:, :], in_=ds[:])

    nc.compile()
    return nc


def _get_module(npc=NPC, mm_dtype_name="float16"):
    key = (npc, mm_dtype_name)
    if key not in _module_cache:
        _module_cache[key] = _build_module(npc, mm_dtype_name)
    return _module_cache[key]


def _prep_in_maps(x, W1, W2, W3, b3, W4, b4, npc=NPC, mm_np_dtype=np.float16):
    """Layout-only host prep: slice per core, transpose so the contraction
    index is the SBUF partition dim, pack per-node small vectors."""
    x = np.asarray(x, np.float32).reshape(1, N)
    W1 = np.asarray(W1, np.float32)
    W2 = np.asarray(W2, np.float32)
    W3 = np.asarray(W3, np.float32)
    b3 = np.asarray(b3, np.float32)
    W4 = np.asarray(W4, np.float32)
    b4 = np.asarray(b4, np.float32).reshape(N, 1)

    ar = np.arange(N)
    W1T = np.ascontiguousarray(W1.transpose(0, 2, 1), dtype=mm_np_dtype)
    W2T = np.ascontiguousarray(W2.transpose(0, 2, 1), dtype=mm_np_dtype)
    W3T = np.ascontiguousarray(W3[:, :, :M].transpose(0, 2, 1), dtype=mm_np_dtype)
    w1d = W1[ar, :, ar]  # [N, 1024]
    w3d = W3[ar, :, M + ar]  # [N, 1024]
    w4s = W4[:, 0, :]  # [N, 1024]

    def colmajor8(a):  # [n, 1024] -> [n, 128, 8] with (p, t) = a[:, t*128+p]
        return a.reshape(-1, 8, 128).transpose(0, 2, 1)

    aux = np.ascontiguousarray(
        np.concatenate(
            [colmajor8(w1d), colmajor8(w3d), colmajor8(b3), colmajor8(w4s)], axis=2
        ),
        dtype=np.float32,
    )  # [N, 128, 32]
    xcv = np.zeros((128, 3), mm_np_dtype)
    xcv[:, 0:2] = x.reshape(2, 128).T.astype(mm_np_dtype)

    n_cores_used = N // npc
    in_maps = []
    for c in range(n_cores_used):
        sl = slice(npc * c, npc * (c + 1))
        in_maps.append(
            {
                "w1t": W1T[sl],
                "w2t": W2T[sl],
                "w3t": W3T[sl],
                "aux": aux[sl],
                "xc": xcv,
                "xn": np.ascontiguousarray(x[:, sl]),
                "b4s": np.ascontiguousarray(b4[sl]),
            }
        )
    return in_maps


def kernel(x, W1, W2, W3, b3, W4, b4, t=0, **_unused):
    from concourse.bass_utils import run_bass_kernel_spmd

    nc = _get_module()
    in_maps = _prep_in_maps(x, W1, W2, W3, b3, W4, b4)
    res = run_bass_kernel_spmd(nc, in_maps, core_ids=list(range(N_CORES)))
    out = np.concatenate([res.results[c]["out"][:, 0] for c in range(N_CORES)])
    kernel.last_results = res
    return np.ascontiguousarray(out.reshape(1, N)).astype(np.float32)


# revision 6
# speedup vs baseline: 2.5979x; 1.0354x over previous
"""Trainium2 Bass kernel for nn_CausalTrajectoryPrediction.

Math (per node n, from the reference):
  A1[n,h]  = <W1[n,h,:], x> - x_n * W1[n,h,n]        (x with x_n zeroed)
  r1       = relu(A1)
  r2[n,m]  = relu(<W2[n,m,:], r1>)
  A3[n,k]  = <W3[n,k,:256], r2> + x_n * W3[n,k,256+n] + b3[n,k]
  h3       = relu(A3)
  d[n]     = relu(<W4[n,0,:], h3> + b4[n])
Only W3[:, :, :256] plus its per-node diagonal column is ever used, so the
second half of W3 (minus the diagonal) is never read from HBM.

Sharding: nodes 32*c..32*c+32 on core c (expert parallel). Host-side prep is
layout-only: slicing, transposes (so the contraction index lands on SBUF
partitions), and packing of the tiny per-node vectors. All FLOPs run on
device: each stage is a chain of accumulating 128x128 @ 128x1 PE matvecs.
"""

import numpy as np

N_CORES = 8
N, H, M = 256, 1024, 256
NPC = N // N_CORES  # 32 nodes per core

_module_cache = {}


def _build_module(npc, mm_dtype_name="float16"):
    import concourse.bacc as bacc
    import concourse.tile as tile
    from concourse import mybir

    f32 = mybir.dt.float32
    mmdt = getattr(mybir.dt, mm_dtype_name)
    AF = mybir.ActivationFunctionType
    OP = mybir.AluOpType

    nc = bacc.Bacc("TRN2", target_bir_lowering=False, debug=False)

    w1t = nc.dram_tensor("w1t", [npc, 256, 1024], mmdt, kind="ExternalInput")
    w2t = nc.dram_tensor("w2t", [npc, 1024, 256], mmdt, kind="ExternalInput")
    w3t = nc.dram_tensor("w3t", [npc, 256, 1024], mmdt, kind="ExternalInput")
    aux = nc.dram_tensor("aux", [npc, 128, 32], f32, kind="ExternalInput")
    xc = nc.dram_tensor("xc", [128, 3], mmdt, kind="ExternalInput")
    xn = nc.dram_tensor("xn", [1, npc], f32, kind="ExternalInput")
    b4s = nc.dram_tensor("b4s", [npc, 1], f32, kind="ExternalInput")
    out = nc.dram_tensor("out", [npc, 1], f32, kind="ExternalOutput")

    with tile.TileContext(nc) as tc:
        with (
            tc.tile_pool(name="singles", bufs=1) as singles,
            tc.tile_pool(name="wpool", bufs=7) as wpool,
            tc.tile_pool(name="auxp", bufs=7) as auxp,
            tc.tile_pool(name="vec", bufs=7) as vec,
            tc.tile_pool(name="psum", bufs=2, space="PSUM") as psum,
            tc.tile_pool(name="psum_d", bufs=1, space="PSUM") as psum_d,
        ):
            xc_sb = singles.tile([128, 3], mmdt)
            nc.sync.dma_start(out=xc_sb[:], in_=xc[:, :])

            # broadcast x_n values across all partitions: [128, npc]
            import concourse.bass as bass

            xn_ap = xn[:, :]
            xn_b = bass.AP(
                tensor=xn_ap.tensor,
                offset=xn_ap.offset,
                ap=[[0, 128]] + [list(d) for d in xn_ap.ap[1:]],
            )
            xnb = singles.tile([128, npc], f32)
            nc.gpsimd.dma_start(out=xnb[:], in_=xn_b)
            xnegb = singles.tile([128, npc], f32)
            nc.vector.tensor_scalar_mul(out=xnegb[:], in0=xnb[:], scalar1=-1.0)

            ones_col = singles.tile([128, 2], f32)
            nc.vector.memset(ones_col[:], 1.0)
            zero1 = singles.tile([128, 1], f32)
            nc.vector.memset(zero1[:], 0.0)
            b4sb = singles.tile([npc, 1], f32)
            nc.scalar.dma_start(out=b4sb[:], in_=b4s[:, :])
            pp = singles.tile([128, npc], f32)

            def emit_load(l):
                w1 = wpool.tile([128, 2, 1024], mmdt, tag="w1")
                nc.sync.dma_start(
                    out=w1[:], in_=w1t[l, :, :].rearrange("(q p) h -> p q h", p=128)
                )
                w2 = wpool.tile([128, 8, 256], mmdt, tag="w2")
                nc.sync.dma_start(
                    out=w2[:], in_=w2t[l, :, :].rearrange("(t p) m -> p t m", p=128)
                )
                w3 = wpool.tile([128, 2, 1024], mmdt, tag="w3")
                nc.sync.dma_start(
                    out=w3[:], in_=w3t[l, :, :].rearrange("(q p) k -> p q k", p=128)
                )
                ax = auxp.tile([128, 32], f32, tag="ax")
                nc.scalar.dma_start(out=ax[:], in_=aux[l, :, :])
                return w1, w2, w3, ax

            def emit_s1(l, w1, ax):
                # S1: A1 chunks t: sum over j-chunks q
                a1p = psum.tile([128, 8, 2], f32, tag="a1")
                for t in range(8):
                    for q in range(2):
                        nc.tensor.matmul(
                            out=a1p[:, t, :],
                            lhsT=w1[:, q, t * 128 : (t + 1) * 128],
                            rhs=xc_sb[:, q : q + 2],
                            start=(q == 0),
                            stop=(q == 1),
                        )
                # a1s = a1p - x_n * w1diag ; relu
                a1s = vec.tile([128, 8], f32, tag="a1s")
                nc.vector.tensor_scalar_mul(
                    out=a1s[:], in0=ax[:, 0:8], scalar1=xnegb[:, l : l + 1]
                )
                nc.vector.tensor_add(out=a1s[:], in0=a1s[:], in1=a1p[:, :, 0])
                r1c = vec.tile([128, 9], mmdt, tag="r1c")
                nc.scalar.activation(out=r1c[:, 0:8], in_=a1s[:], func=AF.Relu)
                nc.scalar.activation(out=r1c[:, 8:9], in_=zero1[:], func=AF.Relu)
                return r1c

            def emit_s2(l, w2, r1c):
                # S2: r2 chunks q: sum over h-chunks t
                a2p = psum.tile([128, 2, 2], f32, tag="a2")
                for q in range(2):
                    for t in range(8):
                        nc.tensor.matmul(
                            out=a2p[:, q, :],
                            lhsT=w2[:, t, q * 128 : (q + 1) * 128],
                            rhs=r1c[:, t : t + 2],
                            start=(t == 0),
                            stop=(t == 7),
                        )
                r2c = vec.tile([128, 3], mmdt, tag="r2c")
                nc.scalar.activation(out=r2c[:, 0:2], in_=a2p[:, :, 0], func=AF.Relu)
                nc.scalar.activation(out=r2c[:, 2:3], in_=zero1[:], func=AF.Relu)
                return r2c

            def emit_s3_s4(l, w3, ax, r2c):
                # S3: A3 chunks t: sum over m-chunks q
                a3p = psum.tile([128, 8, 2], f32, tag="a3")
                for t in range(8):
                    for q in range(2):
                        nc.tensor.matmul(
                            out=a3p[:, t, :],
                            lhsT=w3[:, q, t * 128 : (t + 1) * 128],
                            rhs=r2c[:, q : q + 2],
                            start=(q == 0),
                            stop=(q == 1),
                        )
                # h3 = relu(a3p + x_n * w3diag + b3)
                a3s = vec.tile([128, 8], f32, tag="a3s")
                nc.vector.tensor_scalar_mul(
                    out=a3s[:], in0=ax[:, 8:16], scalar1=xnb[:, l : l + 1]
                )
                nc.vector.tensor_add(out=a3s[:], in0=a3s[:], in1=a3p[:, :, 0])
                nc.vector.tensor_add(out=a3s[:], in0=a3s[:], in1=ax[:, 16:24])
                h3 = vec.tile([128, 8], f32, tag="h3")
                nc.scalar.activation(out=h3[:], in_=a3s[:], func=AF.Relu)

                # S4 partial dot: pp[:, l] = sum_f w4t * h3 (per partition)
                t4 = vec.tile([128, 8], f32, tag="t4")
                nc.vector.tensor_mul(out=t4[:], in0=ax[:, 24:32], in1=h3[:])
                nc.vector.tensor_reduce(
                    pp[:, l : l + 1], t4[:], mybir.AxisListType.X, OP.add
                )

            # software pipeline: S1 at i, S2 at i-2, S3/S4 at i-4
            state = {}
            for i in range(npc + 4):
                if i < npc:
                    w1, w2, w3, ax = emit_load(i)
                    r1c = emit_s1(i, w1, ax)
                    state[i] = [w2, w3, ax, r1c, None]
                if 2 <= i < npc + 2:
                    st = state[i - 2]
                    st[4] = emit_s2(i - 2, st[0], st[3])
                if 4 <= i < npc + 4:
                    st = state.pop(i - 4)
                    emit_s3_s4(i - 4, st[1], st[2], st[4])

            # d = relu(colsum(pp) + b4)
            dp = psum_d.tile([npc, 2], f32, tag="d")
            nc.tensor.matmul(
                out=dp[:], lhsT=pp[:, 0:npc], rhs=ones_col[:], start=True, stop=True
            )
            ds = vec.tile([npc, 1], f32, tag="ds")
            nc.vector.tensor_add(out=ds[:], in0=dp[:, 0:1], in1=b4sb[:])
            nc.scalar.activation(out=ds[:], in_=ds[:], func=AF.Relu)
            nc.sync.dma_start(out=out[:, :], in_=ds[:])

    nc.compile()
    return nc


def _get_module(npc=NPC, mm_dtype_name="float16"):
    key = (npc, mm_dtype_name)
    if key not in _module_cache:
        _module_cache[key] = _build_module(npc, mm_dtype_name)
    return _module_cache[key]


def _prep_in_maps(x, W1, W2, W3, b3, W4, b4, npc=NPC, mm_np_dtype=np.float16):
    """Layout-only host prep: slice per core, transpose so the contraction
    index is the SBUF partition dim, pack per-node small vectors."""
    x = np.asarray(x, np.float32).reshape(1, N)
    W1 = np.asarray(W1, np.float32)
    W2 = np.asarray(W2, np.float32)
    W3 = np.asarray(W3, np.float32)
    b3 = np.asarray(b3, np.float32)
    W4 = np.asarray(W4, np.float32)
    b4 = np.asarray(b4, np.float32).reshape(N, 1)

    ar = np.arange(N)
    W1T = np.ascontiguousarray(W1.transpose(0, 2, 1), dtype=mm_np_dtype)
    W2T = np.ascontiguousarray(W2.transpose(0, 2, 1), dtype=mm_np_dtype)
    W3T = np.ascontiguousarray(W3[:, :, :M].transpose(0, 2, 1), dtype=mm_np_dtype)
    w1d = W1[ar, :, ar]  # [N, 1024]
    w3d = W3[ar, :, M + ar]  # [N, 1024]
    w4s = W4[:, 0, :]  # [N, 1024]

    def colmajor8(a):  # [n, 1024] -> [n, 128, 8] with (p, t) = a[:, t*128+p]
        return a.reshape(-1, 8, 128).transpose(0, 2, 1)

    aux = np.ascontiguousarray(
        np.concatenate(
            [colmajor8(w1d), colmajor8(w3d), colmajor8(b3), colmajor8(w4s)], axis=2
        ),
        dtype=np.float32,
    )  # [N, 128, 32]
    xcv = np.zeros((128, 3), mm_np_dtype)
    xcv[:, 0:2] = x.reshape(2, 128).T.astype(mm_np_dtype)

    n_cores_used = N // npc
    in_maps = []
    for c in range(n_cores_used):
        sl = slice(npc * c, npc * (c + 1))
        in_maps.append(
            {
                "w1t": W1T[sl],
                "w2t": W2T[sl],
                "w3t": W3T[sl],
                "aux": aux[sl],
                "xc": xcv,
                "xn": np.ascontiguousarray(x[:, sl]),
                "b4s": np.ascontiguousarray(b4[sl]),
            }
        )
    return in_maps


def kernel(x, W1, W2, W3, b3, W4, b4, t=0, **_unused):
    from concourse.bass_utils import run_bass_kernel_spmd

    nc = _get_module()
    in_maps = _prep_in_maps(x, W1, W2, W3, b3, W4, b4)
    res = run_bass_kernel_spmd(nc, in_maps, core_ids=list(range(N_CORES)))
    out = np.concatenate([res.results[c]["out"][:, 0] for c in range(N_CORES)])
    kernel.last_results = res
    return np.ascontiguousarray(out.reshape(1, N)).astype(np.float32)


# revision 8
# speedup vs baseline: 2.9724x; 1.1442x over previous
"""Trainium2 Bass kernel for nn_CausalTrajectoryPrediction.

Math (per node n, from the reference):
  A1[n,h]  = <W1[n,h,:], x> - x_n * W1[n,h,n]        (x with x_n zeroed)
  r1       = relu(A1)
  r2[n,m]  = relu(<W2[n,m,:], r1>)
  A3[n,k]  = <W3[n,k,:256], r2> + x_n * W3[n,k,256+n] + b3[n,k]
  h3       = relu(A3)
  d[n]     = relu(<W4[n,0,:], h3> + b4[n])
Only W3[:, :, :256] plus its per-node diagonal column is ever used, so the
second half of W3 (minus the diagonal) is never read from HBM.

Sharding: nodes 32*c..32*c+32 on core c (expert parallel). Host-side prep is
layout-only: slicing, transposes (so the contraction index lands on SBUF
partitions), and packing of the tiny per-node vectors. All FLOPs run on
device: each stage is a chain of accumulating 128x128 @ 128x1 PE matvecs.
"""

import numpy as np

N_CORES = 8
N, H, M = 256, 1024, 256
NPC = N // N_CORES  # 32 nodes per core

_module_cache = {}


def _build_module(npc, mm_dtype_name="float16"):
    import concourse.bacc as bacc
    import concourse.tile as tile
    from concourse import mybir

    f32 = mybir.dt.float32
    mmdt = getattr(mybir.dt, mm_dtype_name)
    AF = mybir.ActivationFunctionType
    OP = mybir.AluOpType

    nc = bacc.Bacc("TRN2", target_bir_lowering=False, debug=False)

    wall = nc.dram_tensor("wall", [npc, 128, 6144], mmdt, kind="ExternalInput")
    aux = nc.dram_tensor("aux", [npc, 128, 32], f32, kind="ExternalInput")
    xc = nc.dram_tensor("xc", [128, 3], mmdt, kind="ExternalInput")
    xn = nc.dram_tensor("xn", [1, npc], f32, kind="ExternalInput")
    b4s = nc.dram_tensor("b4s", [npc, 1], f32, kind="ExternalInput")
    out = nc.dram_tensor("out", [npc, 1], f32, kind="ExternalOutput")

    with tile.TileContext(nc) as tc:
        with (
            tc.tile_pool(name="singles", bufs=1) as singles,
            tc.tile_pool(name="wpool", bufs=6) as wpool,
            tc.tile_pool(name="auxp", bufs=7) as auxp,
            tc.tile_pool(name="vec", bufs=7) as vec,
            tc.tile_pool(name="psum", bufs=2, space="PSUM") as psum,
            tc.tile_pool(name="psum_d", bufs=1, space="PSUM") as psum_d,
        ):
            xc_sb = singles.tile([128, 3], mmdt)
            nc.sync.dma_start(out=xc_sb[:], in_=xc[:, :])

            # broadcast x_n values across all partitions: [128, npc]
            import concourse.bass as bass

            xn_ap = xn[:, :]
            xn_b = bass.AP(
                tensor=xn_ap.tensor,
                offset=xn_ap.offset,
                ap=[[0, 128]] + [list(d) for d in xn_ap.ap[1:]],
            )
            xnb = singles.tile([128, npc], f32)
            nc.gpsimd.dma_start(out=xnb[:], in_=xn_b)
            xnegb = singles.tile([128, npc], f32)
            nc.vector.tensor_scalar_mul(out=xnegb[:], in0=xnb[:], scalar1=-1.0)

            ones_col = singles.tile([128, 2], f32)
            nc.vector.memset(ones_col[:], 1.0)
            zero1 = singles.tile([128, 1], f32)
            nc.vector.memset(zero1[:], 0.0)
            b4sb = singles.tile([npc, 1], f32)
            nc.scalar.dma_start(out=b4sb[:], in_=b4s[:, :])
            pp = singles.tile([128, npc], f32)

            def emit_load(l):
                w = wpool.tile([128, 6144], mmdt, tag="wall")
                nc.sync.dma_start(out=w[:], in_=wall[l, :, :])
                ax = auxp.tile([128, 32], f32, tag="ax")
                nc.sync.dma_start(out=ax[:], in_=aux[l, :, :])
                return w, w, w, ax

            def emit_s1(l, w1, ax):
                # S1: A1 chunks t: sum over j-chunks q
                a1p = psum.tile([128, 8, 2], f32, tag="a1")
                for t in range(8):
                    for q in range(2):
                        nc.tensor.matmul(
                            out=a1p[:, t, :],
                            lhsT=w1[:, q * 1024 + t * 128 : q * 1024 + (t + 1) * 128],
                            rhs=xc_sb[:, q : q + 2],
                            start=(q == 0),
                            stop=(q == 1),
                        )
                # a1s = a1p - x_n * w1diag ; relu
                a1s = vec.tile([128, 8], f32, tag="a1s")
                nc.vector.tensor_scalar_mul(
                    out=a1s[:], in0=ax[:, 0:8], scalar1=xnegb[:, l : l + 1]
                )
                nc.vector.tensor_add(out=a1s[:], in0=a1s[:], in1=a1p[:, :, 0])
                r1c = vec.tile([128, 9], mmdt, tag="r1c")
                nc.vector.memset(r1c[:, 8:9], 0.0)
                nc.scalar.activation(out=r1c[:, 0:8], in_=a1s[:], func=AF.Relu)
                return r1c

            def emit_s2(l, w2, r1c):
                # S2: r2 chunks q: sum over h-chunks t
                a2p = psum.tile([128, 2, 2], f32, tag="a2")
                for q in range(2):
                    for t in range(8):
                        nc.tensor.matmul(
                            out=a2p[:, q, :],
                            lhsT=w2[:, 2048 + t * 256 + q * 128 : 2048 + t * 256 + (q + 1) * 128],
                            rhs=r1c[:, t : t + 2],
                            start=(t == 0),
                            stop=(t == 7),
                        )
                r2c = vec.tile([128, 3], mmdt, tag="r2c")
                nc.vector.memset(r2c[:, 2:3], 0.0)
                nc.scalar.activation(out=r2c[:, 0:2], in_=a2p[:, :, 0], func=AF.Relu)
                return r2c

            def emit_s3_s4(l, w3, ax, r2c):
                # S3: A3 chunks t: sum over m-chunks q
                a3p = psum.tile([128, 8, 2], f32, tag="a3")
                for t in range(8):
                    for q in range(2):
                        nc.tensor.matmul(
                            out=a3p[:, t, :],
                            lhsT=w3[:, 4096 + q * 1024 + t * 128 : 4096 + q * 1024 + (t + 1) * 128],
                            rhs=r2c[:, q : q + 2],
                            start=(q == 0),
                            stop=(q == 1),
                        )
                # h3 = relu(a3p + x_n * w3diag + b3)
                a3s = vec.tile([128, 8], f32, tag="a3s")
                nc.vector.tensor_scalar_mul(
                    out=a3s[:], in0=ax[:, 8:16], scalar1=xnb[:, l : l + 1]
                )
                nc.vector.tensor_add(out=a3s[:], in0=a3s[:], in1=a3p[:, :, 0])
                nc.vector.tensor_add(out=a3s[:], in0=a3s[:], in1=ax[:, 16:24])
                h3 = vec.tile([128, 8], f32, tag="h3")
                nc.scalar.activation(out=h3[:], in_=a3s[:], func=AF.Relu)

                # S4 partial dot: pp[:, l] = sum_f w4t * h3 (per partition)
                t4 = vec.tile([128, 8], f32, tag="t4")
                nc.vector.tensor_mul(out=t4[:], in0=ax[:, 24:32], in1=h3[:])
                nc.vector.tensor_reduce(
                    pp[:, l : l + 1], t4[:], mybir.AxisListType.X, OP.add
                )

            # software pipeline: S1 at i, S2 at i-2, S3/S4 at i-4
            state = {}
            for i in range(npc + 4):
                if i < npc:
                    w1, w2, w3, ax = emit_load(i)
                    r1c = emit_s1(i, w1, ax)
                    state[i] = [w2, w3, ax, r1c, None]
                if 2 <= i < npc + 2:
                    st = state[i - 2]
                    st[4] = emit_s2(i - 2, st[0], st[3])
                if 4 <= i < npc + 4:
                    st = state.pop(i - 4)
                    emit_s3_s4(i - 4, st[1], st[2], st[4])

            # d = relu(colsum(pp) + b4)
            dp = psum_d.tile([npc, 2], f32, tag="d")
            nc.tensor.matmul(
                out=dp[:], lhsT=pp[:, 0:npc], rhs=ones_col[:], start=True, stop=True
            )
            ds = vec.tile([npc, 1], f32, tag="ds")
            nc.vector.tensor_add(out=ds[:], in0=dp[:, 0:1], in1=b4sb[:])
            nc.scalar.activation(out=ds[:], in_=ds[:], func=AF.Relu)
            nc.sync.dma_start(out=out[:, :], in_=ds[:])

    nc.compile()
    return nc


def _get_module(npc=NPC, mm_dtype_name="float16"):
    key = (npc, mm_dtype_name)
    if key not in _module_cache:
        _module_cache[key] = _build_module(npc, mm_dtype_name)
    return _module_cache[key]


def _prep_in_maps(x, W1, W2, W3, b3, W4, b4, npc=NPC, mm_np_dtype=np.float16):
    """Layout-only host prep: slice per core, transpose so the contraction
    index is the SBUF partition dim, pack per-node small vectors."""
    x = np.asarray(x, np.float32).reshape(1, N)
    W1 = np.asarray(W1, np.float32)
    W2 = np.asarray(W2, np.float32)
    W3 = np.asarray(W3, np.float32)
    b3 = np.asarray(b3, np.float32)
    W4 = np.asarray(W4, np.float32)
    b4 = np.asarray(b4, np.float32).reshape(N, 1)

    ar = np.arange(N)
    # pack all matmul weights per node, partition-major so each SBUF
    # partition's span is one contiguous 12KB DRAM run:
    #   cols 0:2048    W1T (q,h):  [p, q*1024+h] = W1[n, h, q*128+p]
    #   cols 2048:4096 W2T (t,m):  [p, t*256+m]  = W2[n, m, t*128+p]
    #   cols 4096:6144 W3T (q,k):  [p, q*1024+k] = W3[n, k, q*128+p]
    W1T = W1.transpose(0, 2, 1).reshape(N, 2, 128, H).transpose(0, 2, 1, 3)
    W2T = W2.transpose(0, 2, 1).reshape(N, 8, 128, M).transpose(0, 2, 1, 3)
    W3T = W3[:, :, :M].transpose(0, 2, 1).reshape(N, 2, 128, H).transpose(0, 2, 1, 3)
    wallv = np.empty((N, 128, 6144), mm_np_dtype)
    wallv[:, :, 0:2048] = W1T.reshape(N, 128, 2048)
    wallv[:, :, 2048:4096] = W2T.reshape(N, 128, 2048)
    wallv[:, :, 4096:6144] = W3T.reshape(N, 128, 2048)
    w1d = W1[ar, :, ar]  # [N, 1024]
    w3d = W3[ar, :, M + ar]  # [N, 1024]
    w4s = W4[:, 0, :]  # [N, 1024]

    def colmajor8(a):  # [n, 1024] -> [n, 128, 8] with (p, t) = a[:, t*128+p]
        return a.reshape(-1, 8, 128).transpose(0, 2, 1)

    aux = np.ascontiguousarray(
        np.concatenate(
            [colmajor8(w1d), colmajor8(w3d), colmajor8(b3), colmajor8(w4s)], axis=2
        ),
        dtype=np.float32,
    )  # [N, 128, 32]
    xcv = np.zeros((128, 3), mm_np_dtype)
    xcv[:, 0:2] = x.reshape(2, 128).T.astype(mm_np_dtype)

    n_cores_used = N // npc
    in_maps = []
    for c in range(n_cores_used):
        sl = slice(npc * c, npc * (c + 1))
        in_maps.append(
            {
                "wall": wallv[sl],
                "aux": aux[sl],
                "xc": xcv,
                "xn": np.ascontiguousarray(x[:, sl]),
                "b4s": np.ascontiguousarray(b4[sl]),
            }
        )
    return in_maps


def kernel(x, W1, W2, W3, b3, W4, b4, t=0, **_unused):
    from concourse.bass_utils import run_bass_kernel_spmd

    nc = _get_module()
    in_maps = _prep_in_maps(x, W1, W2, W3, b3, W4, b4)
    res = run_bass_kernel_spmd(nc, in_maps, core_ids=list(range(N_CORES)))
    out = np.concatenate([res.results[c]["out"][:, 0] for c in range(N_CORES)])
    kernel.last_results = res
    return np.ascontiguousarray(out.reshape(1, N)).astype(np.float32)


# revision 9
# speedup vs baseline: 3.0857x; 1.0381x over previous
"""Trainium2 Bass kernel for nn_CausalTrajectoryPrediction.

Math (per node n, from the reference):
  A1[n,h]  = <W1[n,h,:], x> - x_n * W1[n,h,n]        (x with x_n zeroed)
  r1       = relu(A1)
  r2[n,m]  = relu(<W2[n,m,:], r1>)
  A3[n,k]  = <W3[n,k,:256], r2> + x_n * W3[n,k,256+n] + b3[n,k]
  h3       = relu(A3)
  d[n]     = relu(<W4[n,0,:], h3> + b4[n])
Only W3[:, :, :256] plus its per-node diagonal column is ever used, so the
second half of W3 (minus the diagonal) is never read from HBM.

Sharding: nodes 32*c..32*c+32 on core c (expert parallel). Host-side prep is
layout-only: slicing, transposes (so the contraction index lands on SBUF
partitions), and packing of the tiny per-node vectors. All FLOPs run on
device: each stage is a chain of accumulating 128x128 @ 128x1 PE matvecs.
"""

import numpy as np

N_CORES = 8
N, H, M = 256, 1024, 256
NPC = N // N_CORES  # 32 nodes per core

_module_cache = {}


def _build_module(npc, mm_dtype_name="float16"):
    import concourse.bacc as bacc
    import concourse.tile as tile
    from concourse import mybir

    f32 = mybir.dt.float32
    mmdt = getattr(mybir.dt, mm_dtype_name)
    AF = mybir.ActivationFunctionType
    OP = mybir.AluOpType

    nc = bacc.Bacc("TRN2", target_bir_lowering=False, debug=False)

    wall = nc.dram_tensor("wall", [npc, 128, 6176], mmdt, kind="ExternalInput")
    xc = nc.dram_tensor("xc", [128, 3], mmdt, kind="ExternalInput")
    xn = nc.dram_tensor("xn", [1, npc], f32, kind="ExternalInput")
    b4s = nc.dram_tensor("b4s", [npc, 1], f32, kind="ExternalInput")
    out = nc.dram_tensor("out", [npc, 1], f32, kind="ExternalOutput")

    with tile.TileContext(nc) as tc:
        with (
            tc.tile_pool(name="singles", bufs=1) as singles,
            tc.tile_pool(name="wpool", bufs=8) as wpool,
            tc.tile_pool(name="vec", bufs=7) as vec,
            tc.tile_pool(name="psum", bufs=2, space="PSUM") as psum,
            tc.tile_pool(name="psum_d", bufs=1, space="PSUM") as psum_d,
        ):
            xc_sb = singles.tile([128, 3], mmdt)
            nc.sync.dma_start(out=xc_sb[:], in_=xc[:, :])

            # broadcast x_n values across all partitions: [128, npc]
            import concourse.bass as bass

            xn_ap = xn[:, :]
            xn_b = bass.AP(
                tensor=xn_ap.tensor,
                offset=xn_ap.offset,
                ap=[[0, 128]] + [list(d) for d in xn_ap.ap[1:]],
            )
            xnb = singles.tile([128, npc], f32)
            nc.gpsimd.dma_start(out=xnb[:], in_=xn_b)
            xnegb = singles.tile([128, npc], f32)
            nc.vector.tensor_scalar_mul(out=xnegb[:], in0=xnb[:], scalar1=-1.0)

            ones_col = singles.tile([128, 2], f32)
            nc.vector.memset(ones_col[:], 1.0)
            zero1 = singles.tile([128, 1], f32)
            nc.vector.memset(zero1[:], 0.0)
            b4sb = singles.tile([npc, 1], f32)
            nc.scalar.dma_start(out=b4sb[:], in_=b4s[:, :])
            pp = singles.tile([128, npc], f32)

            def emit_load(l):
                w = wpool.tile([128, 6176], mmdt, tag="wall")
                nc.sync.dma_start(out=w[:], in_=wall[l, :, :])
                return w, w, w, w

            def emit_s1(l, w1, ax):
                # S1: A1 chunks t: sum over j-chunks q
                a1p = psum.tile([128, 8, 2], f32, tag="a1")
                for t in range(8):
                    for q in range(2):
                        nc.tensor.matmul(
                            out=a1p[:, t, :],
                            lhsT=w1[:, q * 1024 + t * 128 : q * 1024 + (t + 1) * 128],
                            rhs=xc_sb[:, q : q + 2],
                            start=(q == 0),
                            stop=(q == 1),
                        )
                # a1s = a1p - x_n * w1diag ; relu
                a1s = vec.tile([128, 8], f32, tag="a1s")
                nc.vector.tensor_scalar_mul(
                    out=a1s[:], in0=ax[:, 6144:6152], scalar1=xnegb[:, l : l + 1]
                )
                nc.vector.tensor_add(out=a1s[:], in0=a1s[:], in1=a1p[:, :, 0])
                r1c = vec.tile([128, 9], mmdt, tag="r1c")
                nc.vector.memset(r1c[:, 8:9], 0.0)
                nc.scalar.activation(out=r1c[:, 0:8], in_=a1s[:], func=AF.Relu)
                return r1c

            def emit_s2(l, w2, r1c):
                # S2: r2 chunks q: sum over h-chunks t
                a2p = psum.tile([128, 2, 2], f32, tag="a2")
                for q in range(2):
                    for t in range(8):
                        nc.tensor.matmul(
                            out=a2p[:, q, :],
                            lhsT=w2[:, 2048 + t * 256 + q * 128 : 2048 + t * 256 + (q + 1) * 128],
                            rhs=r1c[:, t : t + 2],
                            start=(t == 0),
                            stop=(t == 7),
                        )
                r2c = vec.tile([128, 3], mmdt, tag="r2c")
                nc.vector.memset(r2c[:, 2:3], 0.0)
                nc.scalar.activation(out=r2c[:, 0:2], in_=a2p[:, :, 0], func=AF.Relu)
                return r2c

            def emit_s3_s4(l, w3, ax, r2c):
                # S3: A3 chunks t: sum over m-chunks q
                a3p = psum.tile([128, 8, 2], f32, tag="a3")
                for t in range(8):
                    for q in range(2):
                        nc.tensor.matmul(
                            out=a3p[:, t, :],
                            lhsT=w3[:, 4096 + q * 1024 + t * 128 : 4096 + q * 1024 + (t + 1) * 128],
                            rhs=r2c[:, q : q + 2],
                            start=(q == 0),
                            stop=(q == 1),
                        )
                # h3 = relu(a3p + x_n * w3diag + b3)
                a3s = vec.tile([128, 8], f32, tag="a3s")
                nc.vector.tensor_scalar_mul(
                    out=a3s[:], in0=ax[:, 6152:6160], scalar1=xnb[:, l : l + 1]
                )
                nc.vector.tensor_add(out=a3s[:], in0=a3s[:], in1=a3p[:, :, 0])
                nc.vector.tensor_add(out=a3s[:], in0=a3s[:], in1=ax[:, 6160:6168])
                h3 = vec.tile([128, 8], f32, tag="h3")
                nc.scalar.activation(out=h3[:], in_=a3s[:], func=AF.Relu)

                # S4 partial dot: pp[:, l] = sum_f w4t * h3 (per partition)
                t4 = vec.tile([128, 8], f32, tag="t4")
                nc.vector.tensor_mul(out=t4[:], in0=ax[:, 6168:6176], in1=h3[:])
                nc.vector.tensor_reduce(
                    pp[:, l : l + 1], t4[:], mybir.AxisListType.X, OP.add
                )

            # software pipeline: S1 at i, S2 at i-2, S3/S4 at i-4
            state = {}
            for i in range(npc + 4):
                if i < npc:
                    w1, w2, w3, ax = emit_load(i)
                    r1c = emit_s1(i, w1, ax)
                    state[i] = [w2, w3, ax, r1c, None]
                if 2 <= i < npc + 2:
                    st = state[i - 2]
                    st[4] = emit_s2(i - 2, st[0], st[3])
                if 4 <= i < npc + 4:
                    st = state.pop(i - 4)
                    emit_s3_s4(i - 4, st[1], st[2], st[4])

            # d = relu(colsum(pp) + b4)
            dp = psum_d.tile([npc, 2], f32, tag="d")
            nc.tensor.matmul(
                out=dp[:], lhsT=pp[:, 0:npc], rhs=ones_col[:], start=True, stop=True
            )
            ds = vec.tile([npc, 1], f32, tag="ds")
            nc.vector.tensor_add(out=ds[:], in0=dp[:, 0:1], in1=b4sb[:])
            nc.scalar.activation(out=ds[:], in_=ds[:], func=AF.Relu)
            nc.sync.dma_start(out=out[:, :], in_=ds[:])

    nc.compile()
    return nc


def _get_module(npc=NPC, mm_dtype_name="float16"):
    key = (npc, mm_dtype_name)
    if key not in _module_cache:
        _module_cache[key] = _build_module(npc, mm_dtype_name)
    return _module_cache[key]


def _prep_in_maps(x, W1, W2, W3, b3, W4, b4, npc=NPC, mm_np_dtype=np.float16):
    """Layout-only host prep: slice per core, transpose so the contraction
    index is the SBUF partition dim, pack per-node small vectors."""
    x = np.asarray(x, np.float32).reshape(1, N)
    W1 = np.asarray(W1, np.float32)
    W2 = np.asarray(W2, np.float32)
    W3 = np.asarray(W3, np.float32)
    b3 = np.asarray(b3, np.float32)
    W4 = np.asarray(W4, np.float32)
    b4 = np.asarray(b4, np.float32).reshape(N, 1)

    ar = np.arange(N)
    # pack all matmul weights per node, partition-major so each SBUF
    # partition's span is one contiguous 12KB DRAM run:
    #   cols 0:2048    W1T (q,h):  [p, q*1024+h] = W1[n, h, q*128+p]
    #   cols 2048:4096 W2T (t,m):  [p, t*256+m]  = W2[n, m, t*128+p]
    #   cols 4096:6144 W3T (q,k):  [p, q*1024+k] = W3[n, k, q*128+p]
    W1T = W1.transpose(0, 2, 1).reshape(N, 2, 128, H).transpose(0, 2, 1, 3)
    W2T = W2.transpose(0, 2, 1).reshape(N, 8, 128, M).transpose(0, 2, 1, 3)
    W3T = W3[:, :, :M].transpose(0, 2, 1).reshape(N, 2, 128, H).transpose(0, 2, 1, 3)
    wallv = np.empty((N, 128, 6176), mm_np_dtype)
    wallv[:, :, 0:2048] = W1T.reshape(N, 128, 2048)
    wallv[:, :, 2048:4096] = W2T.reshape(N, 128, 2048)
    wallv[:, :, 4096:6144] = W3T.reshape(N, 128, 2048)
    w1d = W1[ar, :, ar]  # [N, 1024]
    w3d = W3[ar, :, M + ar]  # [N, 1024]
    w4s = W4[:, 0, :]  # [N, 1024]

    def colmajor8(a):  # [n, 1024] -> [n, 128, 8] with (p, t) = a[:, t*128+p]
        return a.reshape(-1, 8, 128).transpose(0, 2, 1)

    wallv[:, :, 6144:6152] = colmajor8(w1d)
    wallv[:, :, 6152:6160] = colmajor8(w3d)
    wallv[:, :, 6160:6168] = colmajor8(b3)
    wallv[:, :, 6168:6176] = colmajor8(w4s)
    xcv = np.zeros((128, 3), mm_np_dtype)
    xcv[:, 0:2] = x.reshape(2, 128).T.astype(mm_np_dtype)

    n_cores_used = N // npc
    in_maps = []
    for c in range(n_cores_used):
        sl = slice(npc * c, npc * (c + 1))
        in_maps.append(
            {
                "wall": wallv[sl],
                "xc": xcv,
                "xn": np.ascontiguousarray(x[:, sl]),
                "b4s": np.ascontiguousarray(b4[sl]),
            }
        )
    return in_maps


def kernel(x, W1, W2, W3, b3, W4, b4, t=0, **_unused):
    from concourse.bass_utils import run_bass_kernel_spmd

    nc = _get_module()
    in_maps = _prep_in_maps(x, W1, W2, W3, b3, W4, b4)
    res = run_bass_kernel_spmd(nc, in_maps, core_ids=list(range(N_CORES)))
    out = np.concatenate([res.results[c]["out"][:, 0] for c in range(N_CORES)])
    kernel.last_results = res
    return np.ascontiguousarray(out.reshape(1, N)).astype(np.float32)
